# revision 6
# baseline (speedup 1.0000x reference)
"""MoE (8 experts, top-2) Trainium2 kernel.

Strategy (per spec sharding_hint): expert parallelism. The host computes the
(cheap) router — logits, softmax, top-2, renormalized combine weights — and
dispatches each token to the cores owning its two experts ("all-to-all token
dispatch by top-k expert id" done at the sharding step, since kernel() holds
the full inputs host-side). Core e runs the expert-e FFN over its gathered
tokens, capacity-padded so all 8 cores run one SPMD program:

    Y = W2[e]^T @ gelu(W1[e]^T @ XT + b1[e])        (feature-major layouts)

Both weight matrices stay fully resident in SBUF (bf16, 128KB/partition), so
the gelu intermediate h never round-trips through DRAM: tokens stream in
256-wide tiles, stage 1 produces h one quarter of I at a time into SBUF, and
stage 2 immediately accumulates that quarter into per-output-block PSUM
regions that live across all four quarters.  All matmuls are bf16 at the
full PE rate (1 row/cycle).  The host then scatter-adds
(Y + b2[e]) * combine back into the output.
"""

import os
import sys

import numpy as np

for _p in ("/opt/trn_rl_repo", "/root/.axon_site/_ro/trn_rl_repo"):
    if os.path.isdir(_p) and _p not in sys.path:
        sys.path.insert(0, _p)

NUM_EXPERTS = 8
TOP_K = 2
B, S, H, I = 4, 4096, 1024, 4096
T = B * S
P = 128
NT = 256           # token tile (moving dim)
C_DEFAULT = 4352   # capacity per expert (seed-0 max count 4302), mult of 256

KH = H // P        # 8 contraction chunks for stage 1
KI = I // P        # 32 i-chunks (stage-1 outputs / stage-2 contraction)
NQ = 4             # quarters of I
QI = KI // NQ      # 8 i-chunks per quarter
OB = H // P        # 8 output row-blocks

_built = {}        # (C, reps) -> nc


def _build(C, reps=1):
    import concourse.bacc as bacc
    import concourse.mybir as mybir
    import concourse.tile as tile
    from concourse._compat import get_trn_type

    f32 = mybir.dt.float32
    bf16 = mybir.dt.bfloat16
    GELU = mybir.ActivationFunctionType.Gelu

    assert C % NT == 0
    ntiles = C // NT

    nc = bacc.Bacc(
        get_trn_type() or "TRN2",
        target_bir_lowering=False,
        debug=False,
        enable_asserts=False,
    )
    xt = nc.dram_tensor("xt", [H, C], bf16, kind="ExternalInput").ap()
    w1 = nc.dram_tensor("w1", [H, I], bf16, kind="ExternalInput").ap()
    b1 = nc.dram_tensor("b1", [I], f32, kind="ExternalInput").ap()
    w2 = nc.dram_tensor("w2", [I, H], bf16, kind="ExternalInput").ap()
    ya = nc.dram_tensor("ya", [H, C], f32, kind="ExternalOutput").ap()

    IQ = I // NQ       # 1024 i-range per quarter

    with tile.TileContext(nc) as tc:
        with (
            tc.tile_pool(name="bias", bufs=1) as bpool,
            tc.tile_pool(name="wp", bufs=2) as wp,
            tc.tile_pool(name="xp", bufs=3) as xp,
            tc.tile_pool(name="hp", bufs=3 * QI) as hp,
            tc.tile_pool(name="yp", bufs=8) as yp,
            tc.tile_pool(name="psy", bufs=4, space="PSUM") as psy,
            tc.tile_pool(name="ps1", bufs=4, space="PSUM") as ps1,
        ):
            b1sb = bpool.tile([P, KI], f32)
            nc.sync.dma_start(b1sb[:], b1.rearrange("(ib p) -> p ib", p=P))

            w1r = w1.rearrange("(ko p) i -> p ko i", p=P)
            w2r = w2.rearrange("(ko p) o -> p ko o", p=P)

            for rep in range(reps):
                w1sb = wp.tile([P, KH, I], bf16, tag="w", name=f"w1_{rep}")
                w2sb = wp.tile([P, KI, H], bf16, tag="w", name=f"w2_{rep}")

                def _load_x(t):
                    xst = xp.tile([P, KH, NT], bf16, tag="x",
                                  name=f"x_{rep}_{t}")
                    nc.sync.dma_start(
                        xst[:],
                        xt[:, t * NT:(t + 1) * NT].rearrange(
                            "(ko p) n -> p ko n", p=P),
                    )
                    return xst

                # staged loads: w1 quarter 0, first x tile, then the rest —
                # so PE starts after ~2.5MB of DMA, not ~25MB
                nc.sync.dma_start(w1sb[:, :, 0:IQ], w1r[:, :, 0:IQ])
                x0 = _load_x(0)
                nc.sync.dma_start(w2sb[:, 0:QI], w2r[:, 0:QI])
                for q in range(1, NQ):
                    nc.sync.dma_start(
                        w1sb[:, :, q * IQ:(q + 1) * IQ],
                        w1r[:, :, q * IQ:(q + 1) * IQ],
                    )
                    nc.sync.dma_start(
                        w2sb[:, q * QI:(q + 1) * QI],
                        w2r[:, q * QI:(q + 1) * QI],
                    )

                for t in range(ntiles):
                    xst = x0 if t == 0 else _load_x(t)
                    # two output row-blocks packed per PSUM bank: bank obp
                    # holds ob=2*obp in cols [0,NT) and ob=2*obp+1 in
                    # [NT,2*NT)
                    ypt = [
                        psy.tile([P, 2 * NT], f32, tag="ps",
                                 name=f"y_{rep}_{t}_{obp}")
                        for obp in range(OB // 2)
                    ]
                    for q in range(NQ):
                        hts = []
                        for ic in range(QI):
                            ib = q * QI + ic
                            ps = ps1.tile([P, NT], f32, tag="s1",
                                          name=f"s1_{rep}_{t}_{ib}")
                            for k in range(KH):
                                nc.tensor.matmul(
                                    ps[:],
                                    lhsT=w1sb[:, k, ib * P:(ib + 1) * P],
                                    rhs=xst[:, k],
                                    start=(k == 0),
                                    stop=(k == KH - 1),
                                )
                            ht = hp.tile([P, NT], bf16, tag="h",
                                         name=f"h_{rep}_{t}_{ib}")
                            nc.scalar.activation(
                                ht[:], ps[:], GELU, bias=b1sb[:, ib:ib + 1]
                            )
                            hts.append(ht)
                        for k in range(QI):
                            kg = q * QI + k
                            for ob in range(OB):
                                half = (ob % 2) * NT
                                nc.tensor.matmul(
                                    ypt[ob // 2][:, half:half + NT],
                                    lhsT=w2sb[:, kg, ob * P:(ob + 1) * P],
                                    rhs=hts[k][:],
                                    start=(q == 0 and k == 0),
                                    stop=(q == NQ - 1 and k == QI - 1),
                                )
                    for obp in range(OB // 2):
                        ys = yp.tile([P, 2 * NT], f32, tag="y",
                                     name=f"ys_{rep}_{t}_{obp}")
                        nc.vector.tensor_copy(ys[:], ypt[obp][:])
                        for j in range(2):
                            ob = 2 * obp + j
                            nc.sync.dma_start(
                                ya[ob * P:(ob + 1) * P,
                                   t * NT:(t + 1) * NT],
                                ys[:, j * NT:(j + 1) * NT],
                            )
    nc.finalize()
    return nc


def _routing(hidden, router_w, router_b):
    """Top-2 routing, bit-matching the jax reference on CPU."""
    import jax
    import jax.numpy as jnp

    cpu = jax.local_devices(backend="cpu")[0]
    with jax.default_device(cpu):
        logits = jnp.einsum("bsh,he->bse", jnp.asarray(hidden),
                            jnp.asarray(router_w)) + jnp.asarray(router_b)
        probs = jax.nn.softmax(logits, axis=-1)
        tkp, tki = jax.lax.top_k(probs, TOP_K)
        tkp = tkp / jnp.sum(tkp, axis=-1, keepdims=True)
        tkp_np = np.asarray(tkp).reshape(T, TOP_K)
        tki_np = np.asarray(tki).reshape(T, TOP_K)
    return tkp_np, tki_np


def _prepare(hidden_states, w1, b1, w2, b2, router_w, router_b):
    """Host-side routing + dispatch: returns (in_maps, C, aux for unshard)."""
    hidden_states = np.ascontiguousarray(hidden_states, dtype=np.float32)
    w1 = np.ascontiguousarray(w1, dtype=np.float32)
    b1 = np.ascontiguousarray(b1, dtype=np.float32)
    w2 = np.ascontiguousarray(w2, dtype=np.float32)
    b2 = np.ascontiguousarray(b2, dtype=np.float32)

    import ml_dtypes

    bf16 = ml_dtypes.bfloat16
    w1_bf = w1.astype(bf16)
    w2_bf = w2.astype(bf16)
    tkp, tki = _routing(hidden_states, router_w, router_b)
    x = hidden_states.reshape(T, H)

    idx_e, prob_e = [], []
    for e in range(NUM_EXPERTS):
        hit = tki == e                       # [T, 2] bool
        idx = np.nonzero(hit.any(axis=1))[0]
        pe = np.where(hit[idx, 0], tkp[idx, 0], tkp[idx, 1]).astype(np.float32)
        idx_e.append(idx)
        prob_e.append(pe)

    maxn = max(len(ix) for ix in idx_e)
    C = C_DEFAULT if maxn <= C_DEFAULT else ((maxn + NT - 1) // NT) * NT

    in_maps = []
    for e in range(NUM_EXPERTS):
        ix = idx_e[e]
        xt = np.zeros((H, C), dtype=bf16)
        xt[:, :len(ix)] = x[ix].T
        in_maps.append({
            "xt": xt,
            "w1": w1_bf[e],
            "b1": b1[e],
            "w2": w2_bf[e],
        })
    return in_maps, C, (idx_e, prob_e, b2)


def _unshard(res, aux):
    idx_e, prob_e, b2 = aux
    out = np.zeros((T, H), dtype=np.float32)
    for e in range(NUM_EXPERTS):
        ix = idx_e[e]
        y = res[e]["ya"][:, :len(ix)].T
        out[ix] += (y + b2[e]) * prob_e[e][:, None]
    return out.reshape(B, S, H)


def kernel(hidden_states, w1, b1, w2, b2, router_w, router_b):
    from concourse import bass_utils

    in_maps, C, aux = _prepare(
        hidden_states, w1, b1, w2, b2, router_w, router_b
    )
    if C not in _built:
        _built[C] = _build(C)
    nc = _built[C]

    res = bass_utils.run_bass_kernel_spmd(
        nc, in_maps, core_ids=list(range(NUM_EXPERTS))
    ).results
    return _unshard(res, aux)


# revision 9
# speedup vs baseline: 402.9717x; 402.9717x over previous
"""MoE (8 experts, top-2) Trainium2 kernel.

Strategy (per spec sharding_hint): expert parallelism. The host computes the
(cheap) router — logits, softmax, top-2, renormalized combine weights — and
dispatches each token to the cores owning its two experts ("all-to-all token
dispatch by top-k expert id" done at the sharding step, since kernel() holds
the full inputs host-side). Core e runs the expert-e FFN over its gathered
tokens, capacity-padded so all 8 cores run one SPMD program:

    Y = W2[e]^T @ gelu(W1[e]^T @ XT + b1[e])        (feature-major layouts)

Both weight matrices stay fully resident in SBUF (bf16, 128KB/partition), so
the gelu intermediate h never round-trips through DRAM: tokens stream in
256-wide tiles, stage 1 produces h one quarter of I at a time into SBUF, and
stage 2 immediately accumulates that quarter into per-output-block PSUM
regions that live across all four quarters.  All matmuls are bf16 at the
full PE rate (1 row/cycle).  The host then scatter-adds
(Y + b2[e]) * combine back into the output.
"""

import os
import sys

import numpy as np

for _p in ("/opt/trn_rl_repo", "/root/.axon_site/_ro/trn_rl_repo"):
    if os.path.isdir(_p) and _p not in sys.path:
        sys.path.insert(0, _p)

NUM_EXPERTS = 8
TOP_K = 2
B, S, H, I = 4, 4096, 1024, 4096
T = B * S
P = 128
NT = 256           # token tile (moving dim)
C_DEFAULT = 4352   # capacity per expert (seed-0 max count 4302), mult of 256

KH = H // P        # 8 contraction chunks for stage 1
KI = I // P        # 32 i-chunks (stage-1 outputs / stage-2 contraction)
NQ = 4             # quarters of I
QI = KI // NQ      # 8 i-chunks per quarter
OB = H // P        # 8 output row-blocks

_built = {}        # (C, reps) -> nc


def _build(C, reps=1, act="Gelu"):
    import concourse.bacc as bacc
    import concourse.mybir as mybir
    import concourse.tile as tile
    from concourse._compat import get_trn_type

    f32 = mybir.dt.float32
    bf16 = mybir.dt.bfloat16
    GELU = getattr(mybir.ActivationFunctionType, act)

    assert C % NT == 0
    ntiles = C // NT

    nc = bacc.Bacc(
        get_trn_type() or "TRN2",
        target_bir_lowering=False,
        debug=False,
        enable_asserts=False,
    )
    xt = nc.dram_tensor("xt", [H, C], bf16, kind="ExternalInput").ap()
    w1 = nc.dram_tensor("w1", [H, I], bf16, kind="ExternalInput").ap()
    b1 = nc.dram_tensor("b1", [I], f32, kind="ExternalInput").ap()
    w2 = nc.dram_tensor("w2", [I, H], bf16, kind="ExternalInput").ap()
    ya = nc.dram_tensor("ya", [H, C], f32, kind="ExternalOutput").ap()

    IQ = I // NQ       # 1024 i-range per quarter

    with tile.TileContext(nc) as tc:
        with (
            tc.tile_pool(name="bias", bufs=1) as bpool,
            tc.tile_pool(name="wp", bufs=2) as wp,
            tc.tile_pool(name="xp", bufs=3) as xp,
            tc.tile_pool(name="hp", bufs=2 * KI) as hp,
            tc.tile_pool(name="yp", bufs=8) as yp,
            tc.tile_pool(name="ps2", bufs=4, space="PSUM") as ps2,
            tc.tile_pool(name="ps1", bufs=4, space="PSUM") as ps1,
        ):
            b1sb = bpool.tile([P, KI], f32)
            nc.sync.dma_start(b1sb[:], b1.rearrange("(ib p) -> p ib", p=P))

            w1r = w1.rearrange("(ko p) i -> p ko i", p=P)
            w2r = w2.rearrange("(ko p) o -> p ko o", p=P)

            for rep in range(reps):
                w1sb = wp.tile([P, KH, I], bf16, tag="w", name=f"w1_{rep}")
                w2sb = wp.tile([P, KI, H], bf16, tag="w", name=f"w2_{rep}")

                def _load_x(t):
                    xst = xp.tile([P, KH, NT], bf16, tag="x",
                                  name=f"x_{rep}_{t}")
                    nc.sync.dma_start(
                        xst[:],
                        xt[:, t * NT:(t + 1) * NT].rearrange(
                            "(ko p) n -> p ko n", p=P),
                    )
                    return xst

                # staged loads: w1 quarter 0, first x tile, then the rest —
                # so PE starts after ~2.5MB of DMA, not ~25MB
                nc.sync.dma_start(w1sb[:, :, 0:IQ], w1r[:, :, 0:IQ])
                x0 = _load_x(0)
                nc.sync.dma_start(w2sb[:, 0:QI], w2r[:, 0:QI])
                for q in range(1, NQ):
                    nc.sync.dma_start(
                        w1sb[:, :, q * IQ:(q + 1) * IQ],
                        w1r[:, :, q * IQ:(q + 1) * IQ],
                    )
                    nc.sync.dma_start(
                        w2sb[:, q * QI:(q + 1) * QI],
                        w2r[:, q * QI:(q + 1) * QI],
                    )

                for t in range(ntiles):
                    xst = x0 if t == 0 else _load_x(t)
                    # stage 1: h[ib] = gelu(W1^T x + b1) for all 32 i-chunks,
                    # kept in SBUF for the whole token tile
                    hts = []
                    for ib in range(KI):
                        ps = ps1.tile([P, NT], f32, tag="s1",
                                      name=f"s1_{rep}_{t}_{ib}")
                        for k in range(KH):
                            nc.tensor.matmul(
                                ps[:],
                                lhsT=w1sb[:, k, ib * P:(ib + 1) * P],
                                rhs=xst[:, k],
                                start=(k == 0),
                                stop=(k == KH - 1),
                            )
                        ht = hp.tile([P, NT], bf16, tag="h",
                                     name=f"h_{rep}_{t}_{ib}")
                        nc.scalar.activation(
                            ht[:], ps[:], GELU, bias=b1sb[:, ib:ib + 1]
                        )
                        hts.append(ht)
                    # stage 2: ob-major so only one PSUM accumulation group
                    # is open per rotating bank
                    for ob in range(OB):
                        ps = ps2.tile([P, NT], f32, tag="s2",
                                      name=f"s2_{rep}_{t}_{ob}")
                        for k in range(KI):
                            nc.tensor.matmul(
                                ps[:],
                                lhsT=w2sb[:, k, ob * P:(ob + 1) * P],
                                rhs=hts[k][:],
                                start=(k == 0),
                                stop=(k == KI - 1),
                            )
                        ys = yp.tile([P, NT], f32, tag="y",
                                     name=f"ys_{rep}_{t}_{ob}")
                        nc.vector.tensor_copy(ys[:], ps[:])
                        nc.sync.dma_start(
                            ya[ob * P:(ob + 1) * P, t * NT:(t + 1) * NT],
                            ys[:],
                        )
    nc.finalize()
    return nc


def _routing(hidden, router_w, router_b):
    """Top-2 routing, bit-matching the jax reference on CPU."""
    import jax
    import jax.numpy as jnp

    cpu = jax.local_devices(backend="cpu")[0]
    with jax.default_device(cpu):
        logits = jnp.einsum("bsh,he->bse", jnp.asarray(hidden),
                            jnp.asarray(router_w)) + jnp.asarray(router_b)
        probs = jax.nn.softmax(logits, axis=-1)
        tkp, tki = jax.lax.top_k(probs, TOP_K)
        tkp = tkp / jnp.sum(tkp, axis=-1, keepdims=True)
        tkp_np = np.asarray(tkp).reshape(T, TOP_K)
        tki_np = np.asarray(tki).reshape(T, TOP_K)
    return tkp_np, tki_np


def _prepare(hidden_states, w1, b1, w2, b2, router_w, router_b):
    """Host-side routing + dispatch: returns (in_maps, C, aux for unshard)."""
    hidden_states = np.ascontiguousarray(hidden_states, dtype=np.float32)
    w1 = np.ascontiguousarray(w1, dtype=np.float32)
    b1 = np.ascontiguousarray(b1, dtype=np.float32)
    w2 = np.ascontiguousarray(w2, dtype=np.float32)
    b2 = np.ascontiguousarray(b2, dtype=np.float32)

    import ml_dtypes

    bf16 = ml_dtypes.bfloat16
    w1_bf = w1.astype(bf16)
    w2_bf = w2.astype(bf16)
    tkp, tki = _routing(hidden_states, router_w, router_b)
    x = hidden_states.reshape(T, H)

    idx_e, prob_e = [], []
    for e in range(NUM_EXPERTS):
        hit = tki == e                       # [T, 2] bool
        idx = np.nonzero(hit.any(axis=1))[0]
        pe = np.where(hit[idx, 0], tkp[idx, 0], tkp[idx, 1]).astype(np.float32)
        idx_e.append(idx)
        prob_e.append(pe)

    maxn = max(len(ix) for ix in idx_e)
    C = C_DEFAULT if maxn <= C_DEFAULT else ((maxn + NT - 1) // NT) * NT

    in_maps = []
    for e in range(NUM_EXPERTS):
        ix = idx_e[e]
        xt = np.zeros((H, C), dtype=bf16)
        xt[:, :len(ix)] = x[ix].T
        in_maps.append({
            "xt": xt,
            "w1": w1_bf[e],
            "b1": b1[e],
            "w2": w2_bf[e],
        })
    return in_maps, C, (idx_e, prob_e, b2)


def _unshard(res, aux):
    idx_e, prob_e, b2 = aux
    out = np.zeros((T, H), dtype=np.float32)
    for e in range(NUM_EXPERTS):
        ix = idx_e[e]
        y = res[e]["ya"][:, :len(ix)].T
        out[ix] += (y + b2[e]) * prob_e[e][:, None]
    return out.reshape(B, S, H)


def kernel(hidden_states, w1, b1, w2, b2, router_w, router_b):
    from concourse import bass_utils

    in_maps, C, aux = _prepare(
        hidden_states, w1, b1, w2, b2, router_w, router_b
    )
    if C not in _built:
        _built[C] = _build(C)
    nc = _built[C]

    res = bass_utils.run_bass_kernel_spmd(
        nc, in_maps, core_ids=list(range(NUM_EXPERTS))
    ).results
    return _unshard(res, aux)
